# revision 9
# baseline (speedup 1.0000x reference)
"""Causal self-attention (B=2, L=2048, E=2048, H=16, HD=128) on 8 trn2 cores.

Sharding: core c = (b, g) with b = c // 4 (batch), g = c % 4 (head group of 4).
Each core computes QKV projection for its 4 heads on its batch, causal
attention with RoPE, and a partial output projection (its heads' slice of
w_proj rows). Host sums the 4 partial projections per batch.

All matmuls run in bf16 with fp32 PSUM accumulation (measured end-to-end
rel. error ~5e-3 vs the fp32 reference).

Key device-side structure (per core):
  - phase 1, per 512-wide l-chunk: q/k/v projections as K-accumulated
    matmuls; rope fused right behind each q/k chunk:
        rot = (q * cs) - shuf16(q * ss)     [2 DVE muls + DVE shuffle + sub]
    Head rows are host-permuted so each rope pair partner is p XOR 16 —
    within a 32-partition quadrant — so the swap is a single DVE
    stream_shuffle (no PE matmul).
  - phase 2: scores computed transposed (sT[j,i] = k_j . q_i) so P@V needs
    no transpose; softmax without max-subtraction (|s| <= ~10); exp tiles
    are accumulated per job into an SBUF tile S on the DVE; ONE all-ones
    matmul per job turns S into the denominator (broadcast across
    partitions); causal masking by skipping upper-triangle blocks + 4
    static diagonal masks; software-pipelined with a 3-deep score-matmul
    lookahead.
  - phase 3: partial out-projection, [f, l] layout, fp16 partials, emitted
    per l-chunk as soon as its 4 heads finish (interleaved into attention).

Startup DMA: (wqk chunk e, x tile e) pairs in exact PE consumption order,
even e on the sync HW queue, odd e on the scalar HW queue; bulk loads
(cs/ss/wv/masks/ones/wout) stream on the gpsimd SW queue behind them.

Device layouts (per core):
  xt    [E=2048, L=2048] bf16   x[b].T  (e on rows)
  wqk   [E, 1024]        bf16   8 col-blocks: q-heads 0..3, k-heads 0..3,
                                head rows perm'd to rope-pair order, transposed
  wv    [E, 512]         bf16   v weights, natural order, transposed
  wout  [512, E]         bf16   w_proj[:, g*512:(g+1)*512].T
  cs,ss [128, L]         bf16   rope cos / (-sin|+sin) tables * 128**-0.25
  masks [128, 4*512]     bf16   causal diagonal-block masks
  ones  [128, 128]       bf16   all-ones (softmax denominator broadcast-sum)
Output:
  out   [E, L] fp16  (partial projection, transposed; host adds + transposes
                      in fp32)
"""

from contextlib import ExitStack

import numpy as np
import ml_dtypes

import concourse.bass as bass
import concourse.mybir as mybir
import concourse.tile as tile
from concourse import bacc
from concourse.bass_utils import run_bass_kernel_spmd

BF16 = ml_dtypes.bfloat16
B, L, E, H, HD = 2, 2048, 2048, 16, 128
G = 4            # head groups (cores per batch)
HPG = H // G     # heads per group = 4
NCORES = 8
NE = E // 128    # 16 e-chunks
NLC = L // 512   # 4 l-chunks of 512
NLT = L // 128   # 16 l-tiles of 128
SCALE = float(128.0 ** -0.25)   # per-operand score scale (q and k each)

FP32 = mybir.dt.float32
BF = mybir.dt.bfloat16

# rope partner is p XOR 16: swap 16-row halves within each 32-row quadrant
SWAP16 = [(i + 16) % 32 for i in range(32)]


def build_nc():
    nc = bacc.Bacc(
        "TRN2",
        target_bir_lowering=False,
        debug=False,
        enable_asserts=False,
        num_devices=NCORES,
    )
    d = {}
    d["xt"] = nc.dram_tensor("xt", [E, L], BF, kind="ExternalInput").ap()
    d["wqk"] = nc.dram_tensor("wqk", [E, 2 * HPG * 128], BF, kind="ExternalInput").ap()
    d["wv"] = nc.dram_tensor("wv", [E, HPG * 128], BF, kind="ExternalInput").ap()
    d["wout"] = nc.dram_tensor("wout", [HPG * 128, E], BF, kind="ExternalInput").ap()
    d["cs"] = nc.dram_tensor("cs", [128, L], BF, kind="ExternalInput").ap()
    d["ss"] = nc.dram_tensor("ss", [128, L], BF, kind="ExternalInput").ap()
    d["masks"] = nc.dram_tensor("masks", [128, 4 * 512], BF, kind="ExternalInput").ap()
    d["ones"] = nc.dram_tensor("ones", [128, 128], BF, kind="ExternalInput").ap()
    d["out"] = nc.dram_tensor("out", [E, L], mybir.dt.float16,
                              kind="ExternalOutput").ap()

    with tile.TileContext(nc) as tc:
        build_kernel(tc, d)
    nc.compile()
    return nc


def build_kernel(tc, d):
    nc = tc.nc
    EXP = mybir.ActivationFunctionType.Exp

    with ExitStack() as ctx:
        const = ctx.enter_context(tc.tile_pool(name="const", bufs=1))
        qkres = ctx.enter_context(tc.tile_pool(name="qkres", bufs=1))
        vres = ctx.enter_context(tc.tile_pool(name="vres", bufs=1))
        yres = ctx.enter_context(tc.tile_pool(name="yres", bufs=1))
        xs = ctx.enter_context(tc.tile_pool(name="xs", bufs=24))
        atile = ctx.enter_context(tc.tile_pool(name="atile", bufs=12))
        pexp = ctx.enter_context(tc.tile_pool(name="pexp", bufs=10))
        spool = ctx.enter_context(tc.tile_pool(name="spool", bufs=3))
        zpool = ctx.enter_context(tc.tile_pool(name="zpool", bufs=3))
        outst = ctx.enter_context(tc.tile_pool(name="outst", bufs=2))

        # ---- constants / weights ----
        # wqk chunks + lc0 x tiles are emitted in exact PE consumption order,
        # split across the two hardware DMA queues (sync: even e, scalar:
        # odd e) inside the phase-1 loop below.  Bulk loads (rope tables, wv,
        # masks, wout) are queued BEHIND the lc0 pairs on the same two hw
        # queues so they don't steal DMA bandwidth from the startup-critical
        # path.  Tiles allocated here; DMAs emitted after the lc0 loop.
        wqk_sb = const.tile([128, NE, 2 * HPG * 128], BF, name="wqk_sb", tag="wqk_sb")
        wqk_r = d["wqk"].rearrange("(ec p) f -> p ec f", p=128)
        cs_sb = const.tile([128, L], BF, name="cs_sb", tag="cs_sb")
        ss_sb = const.tile([128, L], BF, name="ss_sb", tag="ss_sb")
        wv_sb = const.tile([128, NE, HPG * 128], BF, name="wv_sb", tag="wv_sb")
        wv_r = d["wv"].rearrange("(ec p) f -> p ec f", p=128)
        masks_sb = const.tile([128, 4, 512], BF, name="masks_sb", tag="masks_sb")
        ones_sb = const.tile([128, 128], BF, name="ones_sb", tag="ones_sb")
        wout_sb = const.tile([128, HPG, E], BF, name="wout_sb", tag="wout_sb")

        # ---- residents ----
        q_sb = [qkres.tile([128, L], BF, name=f"q_sb{h}", tag=f"q_sb{h}")
                for h in range(HPG)]
        k_sb = [qkres.tile([128, L], BF, name=f"k_sb{h}", tag=f"k_sb{h}")
                for h in range(HPG)]
        v_sb = vres.tile([128, NLT, HPG * 128], BF, name="v_sb", tag="v_sb")
        y_sb = [yres.tile([128, L], BF, name=f"y_sb{h}", tag=f"y_sb{h}")
                for h in range(HPG)]

        # ================= phase 1: QKV projection + fused rope ============
        with tc.tile_pool(name="psum1", bufs=1, space="PSUM") as ps1:

            def acc_tile(nm):
                return ps1.tile([128, 512], FP32, name=nm, tag="pacc", bufs=8)

            for lc in range(NLC):
                ls_lo = lc * 512
                cs_lc = cs_sb[:, ls_lo:ls_lo + 512]
                ss_lc = ss_sb[:, ls_lo:ls_lo + 512]

                # bulk pieces interleaved between lc0 (w,x) pairs so they use
                # the DMA bandwidth the PE-paced pair stream leaves spare,
                # without delaying any pair past its consumption time.
                # sync (even pairs): ss, cs, masks, ones; scalar (odd
                # pairs): wv in 4-chunk quarters.  All land before first use
                # (rope ~36us, v_pass ~41us, attention ~190us).
                sync_bulk = {
                    2: [(ss_sb[:, 0:1024], d["ss"][:, 0:1024])],
                    4: [(ss_sb[:, 1024:2048], d["ss"][:, 1024:2048])],
                    6: [(cs_sb[:, 0:1024], d["cs"][:, 0:1024])],
                    8: [(cs_sb[:, 1024:2048], d["cs"][:, 1024:2048])],
                    10: [(masks_sb,
                          d["masks"].rearrange("p (r f) -> p r f", r=4))],
                    12: [(ones_sb, d["ones"])],
                }
                scalar_bulk = {
                    5: [(wv_sb[:, 0:4, :], wv_r[:, 0:4, :])],
                    7: [(wv_sb[:, 4:8, :], wv_r[:, 4:8, :])],
                    9: [(wv_sb[:, 8:12, :], wv_r[:, 8:12, :])],
                    11: [(wv_sb[:, 12:16, :], wv_r[:, 12:16, :])],
                }
                xt_t = []
                for e in range(NE):
                    t = xs.tile([128, 512], BF, name=f"xt_{lc}_{e}", tag="xt")
                    eng = nc.sync if e % 2 == 0 else nc.scalar
                    if lc == 0:
                        # startup-critical: (wqk chunk e, x tile e) pairs in
                        # exact consumption order, alternating between the
                        # two hardware DMA queues
                        eng.dma_start(out=wqk_sb[:, e, :], in_=wqk_r[:, e, :])
                    eng.dma_start(
                        out=t,
                        in_=d["xt"][e * 128:(e + 1) * 128, ls_lo:ls_lo + 512])
                    if lc == 0:
                        for dst, src in (sync_bulk if e % 2 == 0
                                         else scalar_bulk).get(e, []):
                            eng.dma_start(out=dst, in_=src)
                    xt_t.append(t)
                if lc == 1:
                    # needed at first projection (~190us); behind lc1's x
                    nc.sync.dma_start(
                        out=wout_sb,
                        in_=d["wout"].rearrange("(h p) f -> p h f", p=128))

                def qk_pass(halves):
                    # projection matmuls for the given f-block halves
                    # (0 = q heads, 1 = k heads); passing both interleaves
                    # them per e-chunk, which halves the weight-chunk arrival
                    # rate the PE needs (used for the DMA-bound first l-chunk)
                    acc = [acc_tile(f"p{half}_{lc}_{h}")
                           for half in halves for h in range(HPG)]
                    for e in range(NE):
                        for i, half in enumerate(halves):
                            for h in range(HPG):
                                fb = half * HPG + h
                                nc.tensor.matmul(
                                    acc[i * HPG + h],
                                    lhsT=wqk_sb[:, e, fb * 128:(fb + 1) * 128],
                                    rhs=xt_t[e],
                                    start=(e == 0), stop=(e == NE - 1))
                    return acc

                def rope_a(acc, which):
                    # a = q*ss (bf16), dst-slice = q*cs ; releases acc
                    a_t = []
                    for h in range(HPG):
                        a = atile.tile([128, 512], BF,
                                       name=f"a_{which}{h}_{lc}", tag="a")
                        nc.vector.tensor_mul(out=a, in0=acc[h], in1=ss_lc)
                        dst = (q_sb if which == "q" else k_sb)[h]
                        nc.vector.tensor_mul(
                            out=dst[:, ls_lo:ls_lo + 512], in0=acc[h], in1=cs_lc)
                        a_t.append(a)
                    return a_t

                def rope_b(a_t, which):
                    # dst -= shuf16(a)   (rope partner lives at p XOR 16)
                    for h in range(HPG):
                        b = atile.tile([128, 512], BF,
                                       name=f"b_{which}{h}_{lc}", tag="a")
                        nc.vector.stream_shuffle(out=b, in_=a_t[h], mask=SWAP16)
                        dst = (q_sb if which == "q" else k_sb)[h]
                        sl = dst[:, ls_lo:ls_lo + 512]
                        nc.vector.tensor_sub(out=sl, in0=sl, in1=b)

                def v_pass():
                    # v pass (x tiles stationary -> natural [l, d] layout)
                    for ls in range(4):
                        lt = lc * 4 + ls
                        accv = acc_tile(f"pv_{lt}")
                        for e in range(NE):
                            nc.tensor.matmul(
                                accv,
                                lhsT=xt_t[e][:, ls * 128:(ls + 1) * 128],
                                rhs=wv_sb[:, e, :],
                                start=(e == 0), stop=(e == NE - 1))
                        nc.scalar.copy(out=v_sb[:, lt, :], in_=accv)

                if lc == 0:
                    acc8 = qk_pass((0, 1))
                    accq, acck = acc8[:HPG], acc8[HPG:]
                    aq = rope_a(accq, "q")
                    ak = rope_a(acck, "k")
                    rope_b(aq, "q")
                    rope_b(ak, "k")
                    v_pass()
                elif lc == NLC - 1:
                    # last chunk: finalize k before the v pass so attention's
                    # first score matmuls aren't gated on the v matmuls
                    accq = qk_pass((0,))
                    aq = rope_a(accq, "q")
                    acck = qk_pass((1,))
                    rope_b(aq, "q")
                    ak = rope_a(acck, "k")
                    rope_b(ak, "k")
                    v_pass()
                else:
                    accq = qk_pass((0,))
                    aq = rope_a(accq, "q")
                    acck = qk_pass((1,))
                    rope_b(aq, "q")
                    ak = rope_a(acck, "k")
                    v_pass()
                    rope_b(ak, "k")

        # ======== phase 2+3: causal attention with interleaved projection ==
        # jobs are ic-major: once all 4 heads finished l-chunk ic, that
        # chunk's output projection is emitted immediately — it fills
        # attention pipeline bubbles and spreads the output DMA.
        with tc.tile_pool(name="psum2", bufs=1, space="PSUM") as ps2:
            jobs = [(h, ic) for ic in range(NLC) for h in range(HPG)]
            steps = [(ji, jb)
                     for ji, (_h, ic) in enumerate(jobs)
                     for jb in range(4 * ic + 4)]
            LA = 3
            pss_map = {}
            psy_map = {}
            s_map = {}

            def emit_s(ji, jb):
                h, ic = jobs[ji]
                # diagonal blocks (r >= 1) have no valid columns below
                # f = 128*r: compute only the valid column range
                r = jb - 4 * ic
                lo = r * 128 if r > 0 else 0
                t = ps2.tile([128, 512], FP32, name=f"pss_{ji}_{jb}",
                             tag="pss", bufs=4)
                nc.tensor.matmul(
                    t[:, lo:],
                    lhsT=k_sb[h][:, jb * 128:(jb + 1) * 128],
                    rhs=q_sb[h][:, ic * 512 + lo:(ic + 1) * 512],
                    start=True, stop=True)
                pss_map[(ji, jb)] = t

            def emit_proj(ft, lc):
                # one [128f, 512l] block of the output projection for chunk
                # lc (all 4 heads accumulated) + its store/DMA
                po = ps2.tile([128, 512], FP32, name=f"po_{ft}_{lc}",
                              tag="po", bufs=2)
                for hh in range(HPG):
                    nc.tensor.matmul(
                        po,
                        lhsT=wout_sb[:, hh, ft * 128:(ft + 1) * 128],
                        rhs=y_sb[hh][:, lc * 512:(lc + 1) * 512],
                        start=(hh == 0), stop=(hh == HPG - 1))
                ot = outst.tile([128, 512], mybir.dt.float16,
                                name=f"ot_{ft}_{lc}", tag="ot", bufs=6)
                if ft % 2 == 0:
                    nc.vector.tensor_copy(out=ot, in_=po)
                else:
                    nc.scalar.copy(out=ot, in_=po)
                if lc == NLC - 1:
                    # tail: drain the final chunk across all three queues
                    eng = (nc.sync, nc.scalar, nc.gpsimd)[ft % 3]
                else:
                    eng = nc.sync if ft % 2 == 0 else nc.scalar
                eng.dma_start(
                    out=d["out"][ft * 128:(ft + 1) * 128,
                                 lc * 512:(lc + 1) * 512],
                    in_=ot)

            proj_q = []   # pending (ft, lc, earliest step idx) proj blocks
            ptr = 0
            for idx, (ji, jb) in enumerate(steps):
                while ptr < len(steps) and ptr <= idx + LA:
                    emit_s(*steps[ptr])
                    ptr += 1
                h, ic = jobs[ji]
                njb = 4 * ic + 4
                if jb == 0:
                    psy_map[ji] = ps2.tile([128, 512], FP32, name=f"psy_{ji}",
                                           tag="psy", bufs=2)
                    s_map[ji] = spool.tile([128, 512], BF, name=f"s_{ji}",
                                           tag="s")
                psy = psy_map[ji]
                s_t = s_map[ji]
                pss = pss_map.pop((ji, jb))
                r = jb - 4 * ic
                lo = r * 128 if r > 0 else 0
                # block 0's exp goes straight into the S accumulator (it is
                # always full-width); later blocks exp into a scratch tile
                # that is then added to S
                if jb == 0:
                    pt = s_t
                else:
                    pt = pexp.tile([128, 512], BF, name=f"pt_{ji}_{jb}",
                                   tag="pexp")
                nc.scalar.activation(out=pt[:, lo:], in_=pss[:, lo:], func=EXP)
                if r >= 0:
                    # diagonal block: only the first 128 columns of the valid
                    # range hold the per-element triangle; the rest are all-1
                    nc.vector.tensor_mul(
                        out=pt[:, lo:lo + 128], in0=pt[:, lo:lo + 128],
                        in1=masks_sb[:, r, lo:lo + 128])
                if jb > 0:
                    nc.vector.tensor_add(out=s_t[:, lo:], in0=s_t[:, lo:],
                                         in1=pt[:, lo:])
                nc.tensor.matmul(psy[:, lo:],
                                 lhsT=v_sb[:, jb, h * 128:(h + 1) * 128],
                                 rhs=pt[:, lo:],
                                 start=(jb == 0), stop=(jb == njb - 1))
                if jb == njb - 1:
                    # one all-ones matmul: Z broadcast across partitions
                    # (allocated from the pss rotation: it frees a PSUM bank
                    # for the projection double-buffer)
                    psz = ps2.tile([128, 512], FP32, name=f"psz_{ji}",
                                   tag="pss", bufs=4)
                    nc.tensor.matmul(psz, lhsT=ones_sb, rhs=s_map.pop(ji),
                                     start=True, stop=True)
                    zv = zpool.tile([128, 512], FP32, name=f"zinv_{ji}",
                                    tag="zinv")
                    nc.vector.reciprocal_approx_fast(out=zv, in_=psz)
                    nc.vector.tensor_mul(
                        out=y_sb[h][:, ic * 512:(ic + 1) * 512],
                        in0=psy_map.pop(ji), in1=zv)
                    if h == HPG - 1:
                        # all heads done for this l-chunk: queue its
                        # projection blocks; they are dribbled out one per
                        # step below, filling the PE during the next chunk's
                        # attention (which is paced by the scalar-engine
                        # exp).  The first block waits 5 steps so the PE
                        # doesn't stall in-order behind the y finalize chain
                        # (psz matmul -> reciprocal -> y mul on the DVE).
                        proj_q.extend((ft, ic, idx + 5) for ft in range(NE))
                if proj_q and proj_q[0][2] <= idx:
                    ft_, lc_, _ = proj_q.pop(0)
                    emit_proj(ft_, lc_)
            while proj_q:
                ft_, lc_, _ = proj_q.pop(0)
                emit_proj(ft_, lc_)


# ------------------------------------------------------------------ host side

# row p of a head's q/k block holds original head element _PERM_IDX[p]:
# rope pair i occupies rows (q*32 + j, q*32 + 16 + j) with i = q*16 + j,
# so the rotation partner of row p is p XOR 16 (stream_shuffle-able).
_PERM_IDX = np.array(
    [2 * ((p // 32) * 16 + p % 16) + (1 if p % 32 >= 16 else 0)
     for p in range(128)])
_PAIR_IDX = (np.arange(128) // 32) * 16 + np.arange(128) % 16
_SIGN = np.where(np.arange(128) % 32 < 16, -1.0, 1.0).astype(np.float32)


def prep_in_maps(x, rope, w_attn, w_proj):
    x = np.asarray(x, np.float32)
    rope = np.asarray(rope, np.float32)
    w_attn = np.asarray(w_attn, np.float32)
    w_proj = np.asarray(w_proj, np.float32)

    sin = rope[:, :, 0]                      # [L, 64]
    cos = rope[:, :, 1]
    cs = (cos.T[_PAIR_IDX, :] * SCALE).astype(BF16)                 # [128, L]
    ss = (sin.T[_PAIR_IDX, :] * _SIGN[:, None] * SCALE).astype(BF16)

    p = np.arange(128)[:, None]
    f = np.arange(512)[None, :]
    masks = np.zeros((128, 4, 512), np.float32)
    for r in range(4):
        masks[:, r, :] = (r * 128 + p <= f).astype(np.float32)
    masks = masks.reshape(128, 4 * 512).astype(BF16)

    ones = np.ones((128, 128), np.float32).astype(BF16)

    xt_b = [np.ascontiguousarray(x[b].T).astype(BF16) for b in range(B)]

    wqk_g, wv_g, wout_g = {}, {}, {}
    for g in range(G):
        heads = [g * HPG + hl for hl in range(HPG)]
        wq = [np.ascontiguousarray(
                 w_attn[h * 128:(h + 1) * 128, :][_PERM_IDX, :].T) for h in heads]
        wk = [np.ascontiguousarray(
                 w_attn[E + h * 128:E + (h + 1) * 128, :][_PERM_IDX, :].T)
              for h in heads]
        wqk_g[g] = np.concatenate(wq + wk, axis=1).astype(BF16)        # [E, 1024]
        wv_g[g] = np.concatenate(
            [w_attn[2 * E + h * 128:2 * E + (h + 1) * 128, :].T for h in heads],
            axis=1).astype(BF16)                                        # [E, 512]
        wout_g[g] = np.ascontiguousarray(
            w_proj[:, g * 512:(g + 1) * 512].T).astype(BF16)            # [512, E]

    in_maps = []
    for c in range(NCORES):
        b, g = divmod(c, G)
        in_maps.append({
            "xt": xt_b[b],
            "wqk": wqk_g[g],
            "wv": wv_g[g],
            "wout": wout_g[g],
            "cs": cs,
            "ss": ss,
            "masks": masks,
            "ones": ones,
        })
    return in_maps


def assemble_output(results):
    out = np.zeros((B, L, E), np.float32)
    for c in range(NCORES):
        b, g = divmod(c, G)
        out[b] += results[c]["out"].T
    return out


_NC = None


def get_nc():
    global _NC
    if _NC is None:
        _NC = build_nc()
    return _NC


def run(x, rope, w_attn, w_proj, trace=False, tmpdir=None):
    nc = get_nc()
    in_maps = prep_in_maps(x, rope, w_attn, w_proj)
    kwargs = {}
    if trace:
        import sys
        import types
        from concourse import bass_utils as _bu
        try:
            from trn_agent_boot.trn_boot import _ntff_profile_via_ctypes
            hook = _ntff_profile_via_ctypes("/opt/axon/libaxon_pjrt.so")
            mod = types.ModuleType("antenv.axon_hooks")
            mod.get_axon_ntff_profile_hook = lambda: hook
            sys.modules["antenv.axon_hooks"] = mod
            _bu.upload_artifacts = lambda dd: dd
        except Exception as e:  # pragma: no cover
            print("trace hook unavailable:", e)
        kwargs = dict(trace=True, tmpdir=tmpdir)
    res = run_bass_kernel_spmd(nc, in_maps, core_ids=list(range(NCORES)), **kwargs)
    return assemble_output(res.results), res


def kernel(x, rope, w_attn, w_proj):
    out, _ = run(x, rope, w_attn, w_proj, trace=False)
    return out


# revision 10
# speedup vs baseline: 1.0754x; 1.0754x over previous
"""Causal self-attention (B=2, L=2048, E=2048, H=16, HD=128) on 8 trn2 cores.

Sharding: core c = (b, g) with b = c // 4 (batch), g = c % 4 (head group of 4).
Each core computes QKV projection for its 4 heads on its batch, causal
attention with RoPE, and a partial output projection (its heads' slice of
w_proj rows). Host sums the 4 partial projections per batch.

All matmuls run in bf16 with fp32 PSUM accumulation (measured end-to-end
rel. error ~5e-3 vs the fp32 reference).

Key device-side structure (per core):
  - phase 1, per 512-wide l-chunk: q/k/v projections as K-accumulated
    matmuls; rope fused right behind each q/k chunk:
        rot = (q * cs) - shuf16(q * ss)     [2 DVE muls + DVE shuffle + sub]
    Head rows are host-permuted so each rope pair partner is p XOR 16 —
    within a 32-partition quadrant — so the swap is a single DVE
    stream_shuffle (no PE matmul).
  - phase 2: scores computed transposed (sT[j,i] = k_j . q_i) so P@V needs
    no transpose; softmax without max-subtraction (|s| <= ~10); exp tiles
    are accumulated per job into an SBUF tile S on the DVE; ONE all-ones
    matmul per job turns S into the denominator (broadcast across
    partitions); causal masking by skipping upper-triangle blocks + 4
    static diagonal masks; software-pipelined with a 3-deep score-matmul
    lookahead.
  - phase 3: partial out-projection, [f, l] layout, fp16 partials, emitted
    per l-chunk as soon as its 4 heads finish (interleaved into attention).

Startup DMA: (wqk chunk e, x tile e) pairs in exact PE consumption order,
even e on the sync HW queue, odd e on the scalar HW queue; bulk loads
(cs/ss/wv/masks/ones/wout) stream on the gpsimd SW queue behind them.

Device layouts (per core):
  xt    [E=2048, L=2048] bf16   x[b].T  (e on rows)
  wqk   [E, 1024]        bf16   8 col-blocks: q-heads 0..3, k-heads 0..3,
                                head rows perm'd to rope-pair order, transposed
  wv    [E, 512]         bf16   v weights, natural order, transposed
  wout  [512, E]         bf16   w_proj[:, g*512:(g+1)*512].T
  cs,ss [128, L]         bf16   rope cos / (-sin|+sin) tables * 128**-0.25
  masks [128, 4*512]     bf16   causal diagonal-block masks
  ones  [128, 128]       bf16   all-ones (softmax denominator broadcast-sum)
Output:
  out   [E, L] fp16  (partial projection, transposed; host adds + transposes
                      in fp32)
"""

from contextlib import ExitStack

import numpy as np
import ml_dtypes

import concourse.bass as bass
import concourse.mybir as mybir
import concourse.tile as tile
from concourse import bacc
from concourse.bass_utils import run_bass_kernel_spmd

BF16 = ml_dtypes.bfloat16
B, L, E, H, HD = 2, 2048, 2048, 16, 128
G = 4            # head groups (cores per batch)
HPG = H // G     # heads per group = 4
NCORES = 8
NE = E // 128    # 16 e-chunks
NLC = L // 512   # 4 l-chunks of 512
NLT = L // 128   # 16 l-tiles of 128
SCALE = float(128.0 ** -0.25)   # per-operand score scale (q and k each)

FP32 = mybir.dt.float32
BF = mybir.dt.bfloat16

# rope partner is p XOR 16: swap 16-row halves within each 32-row quadrant
SWAP16 = [(i + 16) % 32 for i in range(32)]


def build_nc():
    nc = bacc.Bacc(
        "TRN2",
        target_bir_lowering=False,
        debug=False,
        enable_asserts=False,
        num_devices=NCORES,
    )
    d = {}
    d["xt"] = nc.dram_tensor("xt", [E, L], BF, kind="ExternalInput").ap()
    d["wqk"] = nc.dram_tensor("wqk", [E, 2 * HPG * 128], BF, kind="ExternalInput").ap()
    d["wv"] = nc.dram_tensor("wv", [E, HPG * 128], BF, kind="ExternalInput").ap()
    d["wout"] = nc.dram_tensor("wout", [HPG * 128, E], BF, kind="ExternalInput").ap()
    d["cs"] = nc.dram_tensor("cs", [128, L], BF, kind="ExternalInput").ap()
    d["ss"] = nc.dram_tensor("ss", [128, L], BF, kind="ExternalInput").ap()
    d["masks"] = nc.dram_tensor("masks", [128, 4 * 512], BF, kind="ExternalInput").ap()
    d["ones"] = nc.dram_tensor("ones", [128, 128], BF, kind="ExternalInput").ap()
    d["out"] = nc.dram_tensor("out", [E, L], mybir.dt.float16,
                              kind="ExternalOutput").ap()

    with tile.TileContext(nc) as tc:
        build_kernel(tc, d)
    nc.compile()
    return nc


def build_kernel(tc, d):
    nc = tc.nc
    EXP = mybir.ActivationFunctionType.Exp

    with ExitStack() as ctx:
        const = ctx.enter_context(tc.tile_pool(name="const", bufs=1))
        qkres = ctx.enter_context(tc.tile_pool(name="qkres", bufs=1))
        vres = ctx.enter_context(tc.tile_pool(name="vres", bufs=1))
        yres = ctx.enter_context(tc.tile_pool(name="yres", bufs=1))
        xs = ctx.enter_context(tc.tile_pool(name="xs", bufs=24))
        atile = ctx.enter_context(tc.tile_pool(name="atile", bufs=12))
        pexp = ctx.enter_context(tc.tile_pool(name="pexp", bufs=10))
        spool = ctx.enter_context(tc.tile_pool(name="spool", bufs=3))
        zpool = ctx.enter_context(tc.tile_pool(name="zpool", bufs=3))
        outst = ctx.enter_context(tc.tile_pool(name="outst", bufs=2))

        # ---- constants / weights ----
        # wqk chunks + lc0 x tiles are emitted in exact PE consumption order,
        # split across the two hardware DMA queues (sync: even e, scalar:
        # odd e) inside the phase-1 loop below.  Bulk loads (rope tables, wv,
        # masks, wout) are queued BEHIND the lc0 pairs on the same two hw
        # queues so they don't steal DMA bandwidth from the startup-critical
        # path.  Tiles allocated here; DMAs emitted after the lc0 loop.
        wqk_sb = const.tile([128, NE, 2 * HPG * 128], BF, name="wqk_sb", tag="wqk_sb")
        wqk_r = d["wqk"].rearrange("(ec p) f -> p ec f", p=128)
        cs_sb = const.tile([128, L], BF, name="cs_sb", tag="cs_sb")
        ss_sb = const.tile([128, L], BF, name="ss_sb", tag="ss_sb")
        wv_sb = const.tile([128, NE, HPG * 128], BF, name="wv_sb", tag="wv_sb")
        wv_r = d["wv"].rearrange("(ec p) f -> p ec f", p=128)
        masks_sb = const.tile([128, 4, 512], BF, name="masks_sb", tag="masks_sb")
        ones_sb = const.tile([128, 128], BF, name="ones_sb", tag="ones_sb")
        wout_sb = const.tile([128, HPG, E], BF, name="wout_sb", tag="wout_sb")

        # ---- residents ----
        q_sb = [qkres.tile([128, L], BF, name=f"q_sb{h}", tag=f"q_sb{h}")
                for h in range(HPG)]
        k_sb = [qkres.tile([128, L], BF, name=f"k_sb{h}", tag=f"k_sb{h}")
                for h in range(HPG)]
        v_sb = vres.tile([128, NLT, HPG * 128], BF, name="v_sb", tag="v_sb")
        y_sb = [yres.tile([128, L], BF, name=f"y_sb{h}", tag=f"y_sb{h}")
                for h in range(HPG)]

        # ================= phase 1: QKV projection + fused rope ============
        with tc.tile_pool(name="psum1", bufs=1, space="PSUM") as ps1:

            def acc_tile(nm):
                return ps1.tile([128, 512], FP32, name=nm, tag="pacc", bufs=8)

            for lc in range(NLC):
                ls_lo = lc * 512
                cs_lc = cs_sb[:, ls_lo:ls_lo + 512]
                ss_lc = ss_sb[:, ls_lo:ls_lo + 512]

                xt_t = []
                for e in range(NE):
                    t = xs.tile([128, 512], BF, name=f"xt_{lc}_{e}", tag="xt")
                    eng = nc.sync if e % 2 == 0 else nc.scalar
                    if lc == 0:
                        # startup-critical: (wqk chunk e, x tile e) pairs in
                        # exact consumption order, alternating between the
                        # two hardware DMA queues
                        eng.dma_start(out=wqk_sb[:, e, :], in_=wqk_r[:, e, :])
                    eng.dma_start(
                        out=t,
                        in_=d["xt"][e * 128:(e + 1) * 128, ls_lo:ls_lo + 512])
                    xt_t.append(t)
                if lc == 0:
                    # bulk loads, ordered by first use: rope tables
                    # (~t+35us), wv (~t+40us), masks/ones (phase-2 start),
                    # wout (first projection)
                    nc.scalar.dma_start(out=cs_sb, in_=d["cs"])
                    nc.sync.dma_start(out=ss_sb, in_=d["ss"])
                    nc.scalar.dma_start(out=wv_sb, in_=wv_r)
                    nc.sync.dma_start(
                        out=masks_sb,
                        in_=d["masks"].rearrange("p (r f) -> p r f", r=4))
                    nc.sync.dma_start(out=ones_sb, in_=d["ones"])
                    nc.sync.dma_start(
                        out=wout_sb,
                        in_=d["wout"].rearrange("(h p) f -> p h f", p=128))

                def qk_pass(halves):
                    # projection matmuls for the given f-block halves
                    # (0 = q heads, 1 = k heads); passing both interleaves
                    # them per e-chunk, which halves the weight-chunk arrival
                    # rate the PE needs (used for the DMA-bound first l-chunk)
                    acc = [acc_tile(f"p{half}_{lc}_{h}")
                           for half in halves for h in range(HPG)]
                    for e in range(NE):
                        for i, half in enumerate(halves):
                            for h in range(HPG):
                                fb = half * HPG + h
                                nc.tensor.matmul(
                                    acc[i * HPG + h],
                                    lhsT=wqk_sb[:, e, fb * 128:(fb + 1) * 128],
                                    rhs=xt_t[e],
                                    start=(e == 0), stop=(e == NE - 1))
                    return acc

                def rope_a(acc, which):
                    # a = q*ss (bf16), dst-slice = q*cs ; releases acc
                    a_t = []
                    for h in range(HPG):
                        a = atile.tile([128, 512], BF,
                                       name=f"a_{which}{h}_{lc}", tag="a")
                        nc.vector.tensor_mul(out=a, in0=acc[h], in1=ss_lc)
                        dst = (q_sb if which == "q" else k_sb)[h]
                        nc.vector.tensor_mul(
                            out=dst[:, ls_lo:ls_lo + 512], in0=acc[h], in1=cs_lc)
                        a_t.append(a)
                    return a_t

                def rope_b(a_t, which):
                    # dst -= shuf16(a)   (rope partner lives at p XOR 16)
                    for h in range(HPG):
                        b = atile.tile([128, 512], BF,
                                       name=f"b_{which}{h}_{lc}", tag="a")
                        nc.vector.stream_shuffle(out=b, in_=a_t[h], mask=SWAP16)
                        dst = (q_sb if which == "q" else k_sb)[h]
                        sl = dst[:, ls_lo:ls_lo + 512]
                        nc.vector.tensor_sub(out=sl, in0=sl, in1=b)

                def v_pass():
                    # v pass (x tiles stationary -> natural [l, d] layout)
                    for ls in range(4):
                        lt = lc * 4 + ls
                        accv = acc_tile(f"pv_{lt}")
                        for e in range(NE):
                            nc.tensor.matmul(
                                accv,
                                lhsT=xt_t[e][:, ls * 128:(ls + 1) * 128],
                                rhs=wv_sb[:, e, :],
                                start=(e == 0), stop=(e == NE - 1))
                        nc.scalar.copy(out=v_sb[:, lt, :], in_=accv)

                if lc == 0:
                    acc8 = qk_pass((0, 1))
                    accq, acck = acc8[:HPG], acc8[HPG:]
                    aq = rope_a(accq, "q")
                    ak = rope_a(acck, "k")
                    rope_b(aq, "q")
                    rope_b(ak, "k")
                    v_pass()
                elif lc == NLC - 1:
                    # last chunk: finalize k before the v pass so attention's
                    # first score matmuls aren't gated on the v matmuls
                    accq = qk_pass((0,))
                    aq = rope_a(accq, "q")
                    acck = qk_pass((1,))
                    rope_b(aq, "q")
                    ak = rope_a(acck, "k")
                    rope_b(ak, "k")
                    v_pass()
                else:
                    accq = qk_pass((0,))
                    aq = rope_a(accq, "q")
                    acck = qk_pass((1,))
                    rope_b(aq, "q")
                    ak = rope_a(acck, "k")
                    v_pass()
                    rope_b(ak, "k")

        # ======== phase 2+3: causal attention with interleaved projection ==
        # jobs are ic-major: once all 4 heads finished l-chunk ic, that
        # chunk's output projection is emitted immediately — it fills
        # attention pipeline bubbles and spreads the output DMA.
        with tc.tile_pool(name="psum2", bufs=1, space="PSUM") as ps2:
            jobs = [(h, ic) for ic in range(NLC) for h in range(HPG)]
            steps = [(ji, jb)
                     for ji, (_h, ic) in enumerate(jobs)
                     for jb in range(4 * ic + 4)]
            LA = 3
            pss_map = {}
            psy_map = {}
            s_map = {}

            def emit_s(ji, jb):
                h, ic = jobs[ji]
                # diagonal blocks (r >= 1) have no valid columns below
                # f = 128*r: compute only the valid column range
                r = jb - 4 * ic
                lo = r * 128 if r > 0 else 0
                t = ps2.tile([128, 512], FP32, name=f"pss_{ji}_{jb}",
                             tag="pss", bufs=4)
                nc.tensor.matmul(
                    t[:, lo:],
                    lhsT=k_sb[h][:, jb * 128:(jb + 1) * 128],
                    rhs=q_sb[h][:, ic * 512 + lo:(ic + 1) * 512],
                    start=True, stop=True)
                pss_map[(ji, jb)] = t

            def emit_proj(ft, lc):
                # one [128f, 512l] block of the output projection for chunk
                # lc (all 4 heads accumulated) + its store/DMA
                po = ps2.tile([128, 512], FP32, name=f"po_{ft}_{lc}",
                              tag="po", bufs=2)
                for hh in range(HPG):
                    nc.tensor.matmul(
                        po,
                        lhsT=wout_sb[:, hh, ft * 128:(ft + 1) * 128],
                        rhs=y_sb[hh][:, lc * 512:(lc + 1) * 512],
                        start=(hh == 0), stop=(hh == HPG - 1))
                ot = outst.tile([128, 512], mybir.dt.float16,
                                name=f"ot_{ft}_{lc}", tag="ot", bufs=6)
                if ft % 2 == 0:
                    nc.vector.tensor_copy(out=ot, in_=po)
                else:
                    nc.scalar.copy(out=ot, in_=po)
                if lc == NLC - 1:
                    # tail: drain the final chunk across all three queues
                    eng = (nc.sync, nc.scalar, nc.gpsimd)[ft % 3]
                else:
                    eng = nc.sync if ft % 2 == 0 else nc.scalar
                eng.dma_start(
                    out=d["out"][ft * 128:(ft + 1) * 128,
                                 lc * 512:(lc + 1) * 512],
                    in_=ot)

            proj_q = []   # pending (ft, lc, earliest step idx) proj blocks
            ptr = 0
            for idx, (ji, jb) in enumerate(steps):
                while ptr < len(steps) and ptr <= idx + LA:
                    emit_s(*steps[ptr])
                    ptr += 1
                h, ic = jobs[ji]
                njb = 4 * ic + 4
                if jb == 0:
                    psy_map[ji] = ps2.tile([128, 512], FP32, name=f"psy_{ji}",
                                           tag="psy", bufs=2)
                    s_map[ji] = spool.tile([128, 512], BF, name=f"s_{ji}",
                                           tag="s")
                psy = psy_map[ji]
                s_t = s_map[ji]
                pss = pss_map.pop((ji, jb))
                r = jb - 4 * ic
                lo = r * 128 if r > 0 else 0
                # block 0's exp goes straight into the S accumulator (it is
                # always full-width); later blocks exp into a scratch tile
                # that is then added to S
                if jb == 0:
                    pt = s_t
                else:
                    pt = pexp.tile([128, 512], BF, name=f"pt_{ji}_{jb}",
                                   tag="pexp")
                nc.scalar.activation(out=pt[:, lo:], in_=pss[:, lo:], func=EXP)
                if r >= 0:
                    # diagonal block: only the first 128 columns of the valid
                    # range hold the per-element triangle; the rest are all-1
                    nc.vector.tensor_mul(
                        out=pt[:, lo:lo + 128], in0=pt[:, lo:lo + 128],
                        in1=masks_sb[:, r, lo:lo + 128])
                if jb > 0:
                    nc.vector.tensor_add(out=s_t[:, lo:], in0=s_t[:, lo:],
                                         in1=pt[:, lo:])
                nc.tensor.matmul(psy[:, lo:],
                                 lhsT=v_sb[:, jb, h * 128:(h + 1) * 128],
                                 rhs=pt[:, lo:],
                                 start=(jb == 0), stop=(jb == njb - 1))
                if jb == njb - 1:
                    # one all-ones matmul: Z broadcast across partitions
                    # (allocated from the pss rotation: it frees a PSUM bank
                    # for the projection double-buffer)
                    psz = ps2.tile([128, 512], FP32, name=f"psz_{ji}",
                                   tag="pss", bufs=4)
                    nc.tensor.matmul(psz, lhsT=ones_sb, rhs=s_map.pop(ji),
                                     start=True, stop=True)
                    zv = zpool.tile([128, 512], FP32, name=f"zinv_{ji}",
                                    tag="zinv")
                    nc.vector.reciprocal_approx_fast(out=zv, in_=psz)
                    nc.vector.tensor_mul(
                        out=y_sb[h][:, ic * 512:(ic + 1) * 512],
                        in0=psy_map.pop(ji), in1=zv)
                    if h == HPG - 1:
                        # all heads done for this l-chunk: queue its
                        # projection blocks; they are dribbled out one per
                        # step below, filling the PE during the next chunk's
                        # attention (which is paced by the scalar-engine
                        # exp).  The first block waits 5 steps so the PE
                        # doesn't stall in-order behind the y finalize chain
                        # (psz matmul -> reciprocal -> y mul on the DVE).
                        proj_q.extend((ft, ic, idx + 5) for ft in range(NE))
                if proj_q and proj_q[0][2] <= idx:
                    ft_, lc_, _ = proj_q.pop(0)
                    emit_proj(ft_, lc_)
            while proj_q:
                ft_, lc_, _ = proj_q.pop(0)
                emit_proj(ft_, lc_)


# ------------------------------------------------------------------ host side

# row p of a head's q/k block holds original head element _PERM_IDX[p]:
# rope pair i occupies rows (q*32 + j, q*32 + 16 + j) with i = q*16 + j,
# so the rotation partner of row p is p XOR 16 (stream_shuffle-able).
_PERM_IDX = np.array(
    [2 * ((p // 32) * 16 + p % 16) + (1 if p % 32 >= 16 else 0)
     for p in range(128)])
_PAIR_IDX = (np.arange(128) // 32) * 16 + np.arange(128) % 16
_SIGN = np.where(np.arange(128) % 32 < 16, -1.0, 1.0).astype(np.float32)


def prep_in_maps(x, rope, w_attn, w_proj):
    x = np.asarray(x, np.float32)
    rope = np.asarray(rope, np.float32)
    w_attn = np.asarray(w_attn, np.float32)
    w_proj = np.asarray(w_proj, np.float32)

    sin = rope[:, :, 0]                      # [L, 64]
    cos = rope[:, :, 1]
    cs = (cos.T[_PAIR_IDX, :] * SCALE).astype(BF16)                 # [128, L]
    ss = (sin.T[_PAIR_IDX, :] * _SIGN[:, None] * SCALE).astype(BF16)

    p = np.arange(128)[:, None]
    f = np.arange(512)[None, :]
    masks = np.zeros((128, 4, 512), np.float32)
    for r in range(4):
        masks[:, r, :] = (r * 128 + p <= f).astype(np.float32)
    masks = masks.reshape(128, 4 * 512).astype(BF16)

    ones = np.ones((128, 128), np.float32).astype(BF16)

    xt_b = [np.ascontiguousarray(x[b].T).astype(BF16) for b in range(B)]

    wqk_g, wv_g, wout_g = {}, {}, {}
    for g in range(G):
        heads = [g * HPG + hl for hl in range(HPG)]
        wq = [np.ascontiguousarray(
                 w_attn[h * 128:(h + 1) * 128, :][_PERM_IDX, :].T) for h in heads]
        wk = [np.ascontiguousarray(
                 w_attn[E + h * 128:E + (h + 1) * 128, :][_PERM_IDX, :].T)
              for h in heads]
        wqk_g[g] = np.concatenate(wq + wk, axis=1).astype(BF16)        # [E, 1024]
        wv_g[g] = np.concatenate(
            [w_attn[2 * E + h * 128:2 * E + (h + 1) * 128, :].T for h in heads],
            axis=1).astype(BF16)                                        # [E, 512]
        wout_g[g] = np.ascontiguousarray(
            w_proj[:, g * 512:(g + 1) * 512].T).astype(BF16)            # [512, E]

    in_maps = []
    for c in range(NCORES):
        b, g = divmod(c, G)
        in_maps.append({
            "xt": xt_b[b],
            "wqk": wqk_g[g],
            "wv": wv_g[g],
            "wout": wout_g[g],
            "cs": cs,
            "ss": ss,
            "masks": masks,
            "ones": ones,
        })
    return in_maps


def assemble_output(results):
    out = np.zeros((B, L, E), np.float32)
    for c in range(NCORES):
        b, g = divmod(c, G)
        out[b] += results[c]["out"].T
    return out


_NC = None


def get_nc():
    global _NC
    if _NC is None:
        _NC = build_nc()
    return _NC


def run(x, rope, w_attn, w_proj, trace=False, tmpdir=None):
    nc = get_nc()
    in_maps = prep_in_maps(x, rope, w_attn, w_proj)
    kwargs = {}
    if trace:
        import sys
        import types
        from concourse import bass_utils as _bu
        try:
            from trn_agent_boot.trn_boot import _ntff_profile_via_ctypes
            hook = _ntff_profile_via_ctypes("/opt/axon/libaxon_pjrt.so")
            mod = types.ModuleType("antenv.axon_hooks")
            mod.get_axon_ntff_profile_hook = lambda: hook
            sys.modules["antenv.axon_hooks"] = mod
            _bu.upload_artifacts = lambda dd: dd
        except Exception as e:  # pragma: no cover
            print("trace hook unavailable:", e)
        kwargs = dict(trace=True, tmpdir=tmpdir)
    res = run_bass_kernel_spmd(nc, in_maps, core_ids=list(range(NCORES)), **kwargs)
    return assemble_output(res.results), res


def kernel(x, rope, w_attn, w_proj):
    out, _ = run(x, rope, w_attn, w_proj, trace=False)
    return out


# revision 12
# speedup vs baseline: 1.1008x; 1.0236x over previous
"""Causal self-attention (B=2, L=2048, E=2048, H=16, HD=128) on 8 trn2 cores.

Sharding: core c = (b, g) with b = c // 4 (batch), g = c % 4 (head group of 4).
Each core computes QKV projection for its 4 heads on its batch, causal
attention with RoPE, and a partial output projection (its heads' slice of
w_proj rows). Host sums the 4 partial projections per batch.

All matmuls run in bf16 with fp32 PSUM accumulation (measured end-to-end
rel. error ~5e-3 vs the fp32 reference).

Key device-side structure (per core):
  - phase 1, per 512-wide l-chunk: q/k/v projections as K-accumulated
    matmuls; rope fused right behind each q/k chunk:
        rot = (q * cs) - shuf16(q * ss)     [2 DVE muls + DVE shuffle + sub]
    Head rows are host-permuted so each rope pair partner is p XOR 16 —
    within a 32-partition quadrant — so the swap is a single DVE
    stream_shuffle (no PE matmul).
  - phase 2: scores computed transposed (sT[j,i] = k_j . q_i) so P@V needs
    no transpose; softmax without max-subtraction (|s| <= ~10); exp tiles
    are accumulated per job into an SBUF tile S on the DVE; ONE all-ones
    matmul per job turns S into the denominator (broadcast across
    partitions); causal masking by skipping upper-triangle blocks + 4
    static diagonal masks; software-pipelined with a 3-deep score-matmul
    lookahead.
  - phase 3: partial out-projection, [f, l] layout, fp16 partials, emitted
    per l-chunk as soon as its 4 heads finish (interleaved into attention).

Startup DMA: (wqk chunk e, x tile e) pairs in exact PE consumption order,
even e on the sync HW queue, odd e on the scalar HW queue; bulk loads
(cs/ss/wv/masks/ones/wout) stream on the gpsimd SW queue behind them.

Device layouts (per core):
  xt    [E=2048, L=2048] bf16   x[b].T  (e on rows)
  wqk   [E, 1024]        bf16   8 col-blocks: q-heads 0..3, k-heads 0..3,
                                head rows perm'd to rope-pair order, transposed
  wv    [E, 512]         bf16   v weights, natural order, transposed
  wout  [512, E]         bf16   w_proj[:, g*512:(g+1)*512].T
  cs,ss [128, L]         bf16   rope cos / (-sin|+sin) tables * 128**-0.25
  masks [128, 4*512]     bf16   causal diagonal-block masks
  ones  [128, 128]       bf16   all-ones (softmax denominator broadcast-sum)
Output:
  out   [E, L] fp16  (partial projection, transposed; host adds + transposes
                      in fp32)
"""

from contextlib import ExitStack

import numpy as np
import ml_dtypes

import concourse.bass as bass
import concourse.mybir as mybir
import concourse.tile as tile
from concourse import bacc
from concourse.bass_utils import run_bass_kernel_spmd

BF16 = ml_dtypes.bfloat16
B, L, E, H, HD = 2, 2048, 2048, 16, 128
G = 4            # head groups (cores per batch)
HPG = H // G     # heads per group = 4
NCORES = 8
NE = E // 128    # 16 e-chunks
NLC = L // 512   # 4 l-chunks of 512
NLT = L // 128   # 16 l-tiles of 128
SCALE = float(128.0 ** -0.25)   # per-operand score scale (q and k each)

FP32 = mybir.dt.float32
BF = mybir.dt.bfloat16

# rope partner is p XOR 16: swap 16-row halves within each 32-row quadrant
SWAP16 = [(i + 16) % 32 for i in range(32)]


def build_nc():
    nc = bacc.Bacc(
        "TRN2",
        target_bir_lowering=False,
        debug=False,
        enable_asserts=False,
        num_devices=NCORES,
    )
    d = {}
    d["xt"] = nc.dram_tensor("xt", [E, L], BF, kind="ExternalInput").ap()
    d["wqk"] = nc.dram_tensor("wqk", [E, 2 * HPG * 128], BF, kind="ExternalInput").ap()
    d["wv"] = nc.dram_tensor("wv", [E, HPG * 128], BF, kind="ExternalInput").ap()
    d["wout"] = nc.dram_tensor("wout", [HPG * 128, E], BF, kind="ExternalInput").ap()
    d["cs"] = nc.dram_tensor("cs", [128, L], BF, kind="ExternalInput").ap()
    d["ss"] = nc.dram_tensor("ss", [128, L], BF, kind="ExternalInput").ap()
    d["masks"] = nc.dram_tensor("masks", [128, 4 * 512], BF, kind="ExternalInput").ap()
    d["ones"] = nc.dram_tensor("ones", [128, 128], BF, kind="ExternalInput").ap()
    d["out"] = nc.dram_tensor("out", [E, L], mybir.dt.float16,
                              kind="ExternalOutput").ap()

    with tile.TileContext(nc) as tc:
        build_kernel(tc, d)
    nc.compile()
    return nc


def build_kernel(tc, d):
    nc = tc.nc
    EXP = mybir.ActivationFunctionType.Exp

    with ExitStack() as ctx:
        const = ctx.enter_context(tc.tile_pool(name="const", bufs=1))
        qkres = ctx.enter_context(tc.tile_pool(name="qkres", bufs=1))
        vres = ctx.enter_context(tc.tile_pool(name="vres", bufs=1))
        yres = ctx.enter_context(tc.tile_pool(name="yres", bufs=1))
        xs = ctx.enter_context(tc.tile_pool(name="xs", bufs=24))
        atile = ctx.enter_context(tc.tile_pool(name="atile", bufs=12))
        pexp = ctx.enter_context(tc.tile_pool(name="pexp", bufs=10))
        spool = ctx.enter_context(tc.tile_pool(name="spool", bufs=3))
        zpool = ctx.enter_context(tc.tile_pool(name="zpool", bufs=3))
        outst = ctx.enter_context(tc.tile_pool(name="outst", bufs=2))

        # ---- constants / weights ----
        # wqk chunks + lc0 x tiles are emitted in exact PE consumption order,
        # split across the two hardware DMA queues (sync: even e, scalar:
        # odd e) inside the phase-1 loop below.  Bulk loads (rope tables, wv,
        # masks, wout) are queued BEHIND the lc0 pairs on the same two hw
        # queues so they don't steal DMA bandwidth from the startup-critical
        # path.  Tiles allocated here; DMAs emitted after the lc0 loop.
        wqk_sb = const.tile([128, NE, 2 * HPG * 128], BF, name="wqk_sb", tag="wqk_sb")
        wqk_r = d["wqk"].rearrange("(ec p) f -> p ec f", p=128)
        cs_sb = const.tile([128, L], BF, name="cs_sb", tag="cs_sb")
        ss_sb = const.tile([128, L], BF, name="ss_sb", tag="ss_sb")
        wv_sb = const.tile([128, NE, HPG * 128], BF, name="wv_sb", tag="wv_sb")
        wv_r = d["wv"].rearrange("(ec p) f -> p ec f", p=128)
        masks_sb = const.tile([128, 4, 512], BF, name="masks_sb", tag="masks_sb")
        ones_sb = const.tile([128, 128], BF, name="ones_sb", tag="ones_sb")
        wout_sb = const.tile([128, HPG, E], BF, name="wout_sb", tag="wout_sb")

        # ---- residents ----
        q_sb = [qkres.tile([128, L], BF, name=f"q_sb{h}", tag=f"q_sb{h}")
                for h in range(HPG)]
        k_sb = [qkres.tile([128, L], BF, name=f"k_sb{h}", tag=f"k_sb{h}")
                for h in range(HPG)]
        v_sb = vres.tile([128, NLT, HPG * 128], BF, name="v_sb", tag="v_sb")
        y_sb = [yres.tile([128, L], BF, name=f"y_sb{h}", tag=f"y_sb{h}")
                for h in range(HPG)]

        # ================= phase 1: QKV projection + fused rope ============
        with tc.tile_pool(name="psum1", bufs=1, space="PSUM") as ps1:

            def acc_tile(nm):
                return ps1.tile([128, 512], FP32, name=nm, tag="pacc", bufs=8)

            for lc in range(NLC):
                ls_lo = lc * 512
                cs_lc = cs_sb[:, ls_lo:ls_lo + 512]
                ss_lc = ss_sb[:, ls_lo:ls_lo + 512]

                xt_t = []
                for e in range(NE):
                    t = xs.tile([128, 512], BF, name=f"xt_{lc}_{e}", tag="xt")
                    eng = nc.sync if e % 2 == 0 else nc.scalar
                    if lc == 0:
                        # startup-critical: (wqk chunk e, x tile e) pairs in
                        # exact consumption order, alternating between the
                        # two hardware DMA queues
                        eng.dma_start(out=wqk_sb[:, e, :], in_=wqk_r[:, e, :])
                    eng.dma_start(
                        out=t,
                        in_=d["xt"][e * 128:(e + 1) * 128, ls_lo:ls_lo + 512])
                    xt_t.append(t)
                if lc == 0:
                    # bulk loads, ordered by first use: rope tables
                    # (~t+35us), wv (~t+40us), masks/ones (phase-2 start),
                    # wout (first projection).  wv is chunked so the e-outer
                    # v_pass can start on chunk 0 while the rest stream in.
                    nc.scalar.dma_start(out=cs_sb, in_=d["cs"])
                    nc.sync.dma_start(out=ss_sb, in_=d["ss"])
                    for e in range(NE):
                        nc.scalar.dma_start(out=wv_sb[:, e, :],
                                            in_=wv_r[:, e, :])
                    nc.sync.dma_start(
                        out=masks_sb,
                        in_=d["masks"].rearrange("p (r f) -> p r f", r=4))
                    nc.sync.dma_start(out=ones_sb, in_=d["ones"])
                    nc.sync.dma_start(
                        out=wout_sb,
                        in_=d["wout"].rearrange("(h p) f -> p h f", p=128))

                def qk_pass(halves):
                    # projection matmuls for the given f-block halves
                    # (0 = q heads, 1 = k heads); passing both interleaves
                    # them per e-chunk, which halves the weight-chunk arrival
                    # rate the PE needs (used for the DMA-bound first l-chunk)
                    acc = [acc_tile(f"p{half}_{lc}_{h}")
                           for half in halves for h in range(HPG)]
                    for e in range(NE):
                        for i, half in enumerate(halves):
                            for h in range(HPG):
                                fb = half * HPG + h
                                nc.tensor.matmul(
                                    acc[i * HPG + h],
                                    lhsT=wqk_sb[:, e, fb * 128:(fb + 1) * 128],
                                    rhs=xt_t[e],
                                    start=(e == 0), stop=(e == NE - 1))
                    return acc

                def rope_a(acc, which):
                    # a = q*ss (bf16), dst-slice = q*cs ; releases acc
                    a_t = []
                    for h in range(HPG):
                        a = atile.tile([128, 512], BF,
                                       name=f"a_{which}{h}_{lc}", tag="a")
                        nc.vector.tensor_mul(out=a, in0=acc[h], in1=ss_lc)
                        dst = (q_sb if which == "q" else k_sb)[h]
                        nc.vector.tensor_mul(
                            out=dst[:, ls_lo:ls_lo + 512], in0=acc[h], in1=cs_lc)
                        a_t.append(a)
                    return a_t

                def rope_b(a_t, which):
                    # dst -= shuf16(a)   (rope partner lives at p XOR 16)
                    for h in range(HPG):
                        b = atile.tile([128, 512], BF,
                                       name=f"b_{which}{h}_{lc}", tag="a")
                        nc.vector.stream_shuffle(out=b, in_=a_t[h], mask=SWAP16)
                        dst = (q_sb if which == "q" else k_sb)[h]
                        sl = dst[:, ls_lo:ls_lo + 512]
                        nc.vector.tensor_sub(out=sl, in0=sl, in1=b)

                def v_pass():
                    # v pass (x tiles stationary -> natural [l, d] layout);
                    # e-outer with 4 live accumulators so the first chunk's
                    # arrival unblocks the whole pass (wv streams chunk-wise)
                    accv = [acc_tile(f"pv_{lc * 4 + ls}") for ls in range(4)]
                    for e in range(NE):
                        for ls in range(4):
                            nc.tensor.matmul(
                                accv[ls],
                                lhsT=xt_t[e][:, ls * 128:(ls + 1) * 128],
                                rhs=wv_sb[:, e, :],
                                start=(e == 0), stop=(e == NE - 1))
                    for ls in range(4):
                        nc.scalar.copy(out=v_sb[:, lc * 4 + ls, :],
                                       in_=accv[ls])

                if lc == 0:
                    acc8 = qk_pass((0, 1))
                    accq, acck = acc8[:HPG], acc8[HPG:]
                    aq = rope_a(accq, "q")
                    ak = rope_a(acck, "k")
                    rope_b(aq, "q")
                    rope_b(ak, "k")
                    v_pass()
                elif lc == NLC - 1:
                    # last chunk: finalize k before the v pass so attention's
                    # first score matmuls aren't gated on the v matmuls
                    accq = qk_pass((0,))
                    aq = rope_a(accq, "q")
                    acck = qk_pass((1,))
                    rope_b(aq, "q")
                    ak = rope_a(acck, "k")
                    rope_b(ak, "k")
                    v_pass()
                else:
                    accq = qk_pass((0,))
                    aq = rope_a(accq, "q")
                    acck = qk_pass((1,))
                    rope_b(aq, "q")
                    ak = rope_a(acck, "k")
                    v_pass()
                    rope_b(ak, "k")

        # ======== phase 2+3: causal attention with interleaved projection ==
        # jobs are ic-major: once all 4 heads finished l-chunk ic, that
        # chunk's output projection is emitted immediately — it fills
        # attention pipeline bubbles and spreads the output DMA.
        with tc.tile_pool(name="psum2", bufs=1, space="PSUM") as ps2:
            jobs = [(h, ic) for ic in range(NLC) for h in range(HPG)]
            steps = [(ji, jb)
                     for ji, (_h, ic) in enumerate(jobs)
                     for jb in range(4 * ic + 4)]
            LA = 3
            pss_map = {}
            psy_map = {}
            s_map = {}

            def emit_s(ji, jb):
                h, ic = jobs[ji]
                # diagonal blocks (r >= 1) have no valid columns below
                # f = 128*r: compute only the valid column range
                r = jb - 4 * ic
                lo = r * 128 if r > 0 else 0
                t = ps2.tile([128, 512], FP32, name=f"pss_{ji}_{jb}",
                             tag="pss", bufs=4)
                nc.tensor.matmul(
                    t[:, lo:],
                    lhsT=k_sb[h][:, jb * 128:(jb + 1) * 128],
                    rhs=q_sb[h][:, ic * 512 + lo:(ic + 1) * 512],
                    start=True, stop=True)
                pss_map[(ji, jb)] = t

            def emit_proj(ft, lc):
                # one [128f, 512l] block of the output projection for chunk
                # lc (all 4 heads accumulated) + its store/DMA
                po = ps2.tile([128, 512], FP32, name=f"po_{ft}_{lc}",
                              tag="po", bufs=2)
                for hh in range(HPG):
                    nc.tensor.matmul(
                        po,
                        lhsT=wout_sb[:, hh, ft * 128:(ft + 1) * 128],
                        rhs=y_sb[hh][:, lc * 512:(lc + 1) * 512],
                        start=(hh == 0), stop=(hh == HPG - 1))
                ot = outst.tile([128, 512], mybir.dt.float16,
                                name=f"ot_{ft}_{lc}", tag="ot", bufs=6)
                if ft % 2 == 0:
                    nc.vector.tensor_copy(out=ot, in_=po)
                else:
                    nc.scalar.copy(out=ot, in_=po)
                if lc == NLC - 1:
                    # tail: drain the final chunk across all three queues
                    eng = (nc.sync, nc.scalar, nc.gpsimd)[ft % 3]
                else:
                    eng = nc.sync if ft % 2 == 0 else nc.scalar
                eng.dma_start(
                    out=d["out"][ft * 128:(ft + 1) * 128,
                                 lc * 512:(lc + 1) * 512],
                    in_=ot)

            proj_q = []   # pending (ft, lc, earliest step idx) proj blocks
            ptr = 0
            for idx, (ji, jb) in enumerate(steps):
                while ptr < len(steps) and ptr <= idx + LA:
                    emit_s(*steps[ptr])
                    ptr += 1
                h, ic = jobs[ji]
                njb = 4 * ic + 4
                if jb == 0:
                    psy_map[ji] = ps2.tile([128, 512], FP32, name=f"psy_{ji}",
                                           tag="psy", bufs=2)
                    s_map[ji] = spool.tile([128, 512], BF, name=f"s_{ji}",
                                           tag="s")
                psy = psy_map[ji]
                s_t = s_map[ji]
                pss = pss_map.pop((ji, jb))
                r = jb - 4 * ic
                lo = r * 128 if r > 0 else 0
                # block 0's exp goes straight into the S accumulator (it is
                # always full-width); later blocks exp into a scratch tile
                # that is then added to S
                if jb == 0:
                    pt = s_t
                else:
                    pt = pexp.tile([128, 512], BF, name=f"pt_{ji}_{jb}",
                                   tag="pexp")
                nc.scalar.activation(out=pt[:, lo:], in_=pss[:, lo:], func=EXP)
                if r >= 0:
                    # diagonal block: only the first 128 columns of the valid
                    # range hold the per-element triangle; the rest are all-1
                    nc.vector.tensor_mul(
                        out=pt[:, lo:lo + 128], in0=pt[:, lo:lo + 128],
                        in1=masks_sb[:, r, lo:lo + 128])
                if jb > 0:
                    nc.vector.tensor_add(out=s_t[:, lo:], in0=s_t[:, lo:],
                                         in1=pt[:, lo:])
                nc.tensor.matmul(psy[:, lo:],
                                 lhsT=v_sb[:, jb, h * 128:(h + 1) * 128],
                                 rhs=pt[:, lo:],
                                 start=(jb == 0), stop=(jb == njb - 1))
                if jb == njb - 1:
                    # one all-ones matmul: Z broadcast across partitions
                    # (allocated from the pss rotation: it frees a PSUM bank
                    # for the projection double-buffer)
                    psz = ps2.tile([128, 512], FP32, name=f"psz_{ji}",
                                   tag="pss", bufs=4)
                    nc.tensor.matmul(psz, lhsT=ones_sb, rhs=s_map.pop(ji),
                                     start=True, stop=True)
                    zv = zpool.tile([128, 512], FP32, name=f"zinv_{ji}",
                                    tag="zinv")
                    nc.vector.reciprocal_approx_fast(out=zv, in_=psz)
                    nc.vector.tensor_mul(
                        out=y_sb[h][:, ic * 512:(ic + 1) * 512],
                        in0=psy_map.pop(ji), in1=zv)
                    if h == HPG - 1:
                        # all heads done for this l-chunk: queue its
                        # projection blocks; they are dribbled out one per
                        # step below, filling the PE during the next chunk's
                        # attention (which is paced by the scalar-engine
                        # exp).  The first block waits 5 steps so the PE
                        # doesn't stall in-order behind the y finalize chain
                        # (psz matmul -> reciprocal -> y mul on the DVE).
                        proj_q.extend((ft, ic, idx + 5) for ft in range(NE))
                if proj_q and proj_q[0][2] <= idx:
                    ft_, lc_, _ = proj_q.pop(0)
                    emit_proj(ft_, lc_)
            while proj_q:
                ft_, lc_, _ = proj_q.pop(0)
                emit_proj(ft_, lc_)


# ------------------------------------------------------------------ host side

# row p of a head's q/k block holds original head element _PERM_IDX[p]:
# rope pair i occupies rows (q*32 + j, q*32 + 16 + j) with i = q*16 + j,
# so the rotation partner of row p is p XOR 16 (stream_shuffle-able).
_PERM_IDX = np.array(
    [2 * ((p // 32) * 16 + p % 16) + (1 if p % 32 >= 16 else 0)
     for p in range(128)])
_PAIR_IDX = (np.arange(128) // 32) * 16 + np.arange(128) % 16
_SIGN = np.where(np.arange(128) % 32 < 16, -1.0, 1.0).astype(np.float32)


def prep_in_maps(x, rope, w_attn, w_proj):
    x = np.asarray(x, np.float32)
    rope = np.asarray(rope, np.float32)
    w_attn = np.asarray(w_attn, np.float32)
    w_proj = np.asarray(w_proj, np.float32)

    sin = rope[:, :, 0]                      # [L, 64]
    cos = rope[:, :, 1]
    cs = (cos.T[_PAIR_IDX, :] * SCALE).astype(BF16)                 # [128, L]
    ss = (sin.T[_PAIR_IDX, :] * _SIGN[:, None] * SCALE).astype(BF16)

    p = np.arange(128)[:, None]
    f = np.arange(512)[None, :]
    masks = np.zeros((128, 4, 512), np.float32)
    for r in range(4):
        masks[:, r, :] = (r * 128 + p <= f).astype(np.float32)
    masks = masks.reshape(128, 4 * 512).astype(BF16)

    ones = np.ones((128, 128), np.float32).astype(BF16)

    xt_b = [np.ascontiguousarray(x[b].T).astype(BF16) for b in range(B)]

    wqk_g, wv_g, wout_g = {}, {}, {}
    for g in range(G):
        heads = [g * HPG + hl for hl in range(HPG)]
        wq = [np.ascontiguousarray(
                 w_attn[h * 128:(h + 1) * 128, :][_PERM_IDX, :].T) for h in heads]
        wk = [np.ascontiguousarray(
                 w_attn[E + h * 128:E + (h + 1) * 128, :][_PERM_IDX, :].T)
              for h in heads]
        wqk_g[g] = np.concatenate(wq + wk, axis=1).astype(BF16)        # [E, 1024]
        wv_g[g] = np.concatenate(
            [w_attn[2 * E + h * 128:2 * E + (h + 1) * 128, :].T for h in heads],
            axis=1).astype(BF16)                                        # [E, 512]
        wout_g[g] = np.ascontiguousarray(
            w_proj[:, g * 512:(g + 1) * 512].T).astype(BF16)            # [512, E]

    in_maps = []
    for c in range(NCORES):
        b, g = divmod(c, G)
        in_maps.append({
            "xt": xt_b[b],
            "wqk": wqk_g[g],
            "wv": wv_g[g],
            "wout": wout_g[g],
            "cs": cs,
            "ss": ss,
            "masks": masks,
            "ones": ones,
        })
    return in_maps


def assemble_output(results):
    out = np.zeros((B, L, E), np.float32)
    for c in range(NCORES):
        b, g = divmod(c, G)
        out[b] += results[c]["out"].T
    return out


_NC = None


def get_nc():
    global _NC
    if _NC is None:
        _NC = build_nc()
    return _NC


def run(x, rope, w_attn, w_proj, trace=False, tmpdir=None):
    nc = get_nc()
    in_maps = prep_in_maps(x, rope, w_attn, w_proj)
    kwargs = {}
    if trace:
        import sys
        import types
        from concourse import bass_utils as _bu
        try:
            from trn_agent_boot.trn_boot import _ntff_profile_via_ctypes
            hook = _ntff_profile_via_ctypes("/opt/axon/libaxon_pjrt.so")
            mod = types.ModuleType("antenv.axon_hooks")
            mod.get_axon_ntff_profile_hook = lambda: hook
            sys.modules["antenv.axon_hooks"] = mod
            _bu.upload_artifacts = lambda dd: dd
        except Exception as e:  # pragma: no cover
            print("trace hook unavailable:", e)
        kwargs = dict(trace=True, tmpdir=tmpdir)
    res = run_bass_kernel_spmd(nc, in_maps, core_ids=list(range(NCORES)), **kwargs)
    return assemble_output(res.results), res


def kernel(x, rope, w_attn, w_proj):
    out, _ = run(x, rope, w_attn, w_proj, trace=False)
    return out


# revision 18
# speedup vs baseline: 1.1618x; 1.0554x over previous
"""Causal self-attention (B=2, L=2048, E=2048, H=16, HD=128) on 8 trn2 cores.

Sharding: core c = (b, g) with b = c // 4 (batch), g = c % 4 (head group of 4).
Each core computes QKV projection for its 4 heads on its batch, causal
attention with RoPE, and a partial output projection (its heads' slice of
w_proj rows). Host sums the 4 partial projections per batch.

All matmuls run in bf16 with fp32 PSUM accumulation (measured end-to-end
rel. error ~5e-3 vs the fp32 reference).

Key device-side structure (per core):
  - phase 1, per 512-wide l-chunk: q/k/v projections as K-accumulated
    matmuls; rope fused right behind each q/k chunk:
        rot = (q * cs) - shuf16(q * ss)     [2 DVE muls + DVE shuffle + sub]
    Head rows are host-permuted so each rope pair partner is p XOR 16 —
    within a 32-partition quadrant — so the swap is a single DVE
    stream_shuffle (no PE matmul).
  - phase 2: scores computed transposed (sT[j,i] = k_j . q_i) so P@V needs
    no transpose; softmax without max-subtraction (|s| <= ~10); exp tiles
    are accumulated per job into an SBUF tile S on the DVE; ONE all-ones
    matmul per job turns S into the denominator (broadcast across
    partitions); causal masking by skipping upper-triangle blocks + 4
    static diagonal masks; software-pipelined with a 3-deep score-matmul
    lookahead.
  - phase 3: partial out-projection, [f, l] layout, fp16 partials, emitted
    per l-chunk as soon as its 4 heads finish (interleaved into attention).

Startup DMA: (wqk chunk e, x tile e) pairs in exact PE consumption order,
even e on the sync HW queue, odd e on the scalar HW queue; bulk loads
(cs/ss/wv/masks/ones/wout) stream on the gpsimd SW queue behind them.

Device layouts (per core):
  xt    [E=2048, L=2048] bf16   x[b].T  (e on rows)
  wqk   [E, 1024]        bf16   8 col-blocks: q-heads 0..3, k-heads 0..3,
                                head rows perm'd to rope-pair order, transposed
  wv    [E, 512]         bf16   v weights, natural order, transposed
  wout  [512, E]         bf16   w_proj[:, g*512:(g+1)*512].T
  cs,ss [128, L]         bf16   rope cos / (-sin|+sin) tables * 128**-0.25
  masks [128, 4*512]     bf16   causal diagonal-block masks
  ones  [128, 128]       bf16   all-ones (softmax denominator broadcast-sum)
Output:
  out   [E, L] fp16  (partial projection, transposed; host adds + transposes
                      in fp32)
"""

from contextlib import ExitStack

import numpy as np
import ml_dtypes

import concourse.bass as bass
import concourse.mybir as mybir
import concourse.tile as tile
from concourse import bacc
from concourse.bass_utils import run_bass_kernel_spmd

BF16 = ml_dtypes.bfloat16
B, L, E, H, HD = 2, 2048, 2048, 16, 128
G = 4            # head groups (cores per batch)
HPG = H // G     # heads per group = 4
NCORES = 8
NE = E // 128    # 16 e-chunks
NLC = L // 512   # 4 l-chunks of 512
NLT = L // 128   # 16 l-tiles of 128
SCALE = float(128.0 ** -0.25)   # per-operand score scale (q and k each)

FP32 = mybir.dt.float32
BF = mybir.dt.bfloat16

# rope partner is p XOR 16: swap 16-row halves within each 32-row quadrant
SWAP16 = [(i + 16) % 32 for i in range(32)]


def build_nc():
    nc = bacc.Bacc(
        "TRN2",
        target_bir_lowering=False,
        debug=False,
        enable_asserts=False,
        num_devices=NCORES,
    )
    d = {}
    d["xt"] = nc.dram_tensor("xt", [E, L], BF, kind="ExternalInput").ap()
    d["wqk"] = nc.dram_tensor("wqk", [E, 2 * HPG * 128], BF, kind="ExternalInput").ap()
    d["wv"] = nc.dram_tensor("wv", [E, HPG * 128], BF, kind="ExternalInput").ap()
    d["wout"] = nc.dram_tensor("wout", [HPG * 128, E], BF, kind="ExternalInput").ap()
    d["cs"] = nc.dram_tensor("cs", [128, L], BF, kind="ExternalInput").ap()
    d["ss"] = nc.dram_tensor("ss", [128, L], BF, kind="ExternalInput").ap()
    d["masks"] = nc.dram_tensor("masks", [128, 4 * 512], BF, kind="ExternalInput").ap()
    d["ones"] = nc.dram_tensor("ones", [128, 128], BF, kind="ExternalInput").ap()
    d["out"] = nc.dram_tensor("out", [E, L], mybir.dt.float16,
                              kind="ExternalOutput").ap()

    with tile.TileContext(nc) as tc:
        build_kernel(tc, d)
    nc.compile()
    return nc


def build_kernel(tc, d):
    nc = tc.nc
    EXP = mybir.ActivationFunctionType.Exp

    with ExitStack() as ctx:
        const = ctx.enter_context(tc.tile_pool(name="const", bufs=1))
        qkres = ctx.enter_context(tc.tile_pool(name="qkres", bufs=1))
        vres = ctx.enter_context(tc.tile_pool(name="vres", bufs=1))
        yres = ctx.enter_context(tc.tile_pool(name="yres", bufs=1))
        xs = ctx.enter_context(tc.tile_pool(name="xs", bufs=32))
        atile = ctx.enter_context(tc.tile_pool(name="atile", bufs=12))
        pexp = ctx.enter_context(tc.tile_pool(name="pexp", bufs=8))
        spool = ctx.enter_context(tc.tile_pool(name="spool", bufs=3))
        zpool = ctx.enter_context(tc.tile_pool(name="zpool", bufs=2))
        outst = ctx.enter_context(tc.tile_pool(name="outst", bufs=2))

        # ---- constants / weights ----
        # wqk chunks + lc0 x tiles are emitted in exact PE consumption order,
        # split across the two hardware DMA queues (sync: even e, scalar:
        # odd e) inside the phase-1 loop below.  Bulk loads (rope tables, wv,
        # masks, wout) are queued BEHIND the lc0 pairs on the same two hw
        # queues so they don't steal DMA bandwidth from the startup-critical
        # path.  Tiles allocated here; DMAs emitted after the lc0 loop.
        wqk_sb = const.tile([128, NE, 2 * HPG * 128], BF, name="wqk_sb", tag="wqk_sb")
        wqk_r = d["wqk"].rearrange("(ec p) f -> p ec f", p=128)
        cs_sb = const.tile([128, L], BF, name="cs_sb", tag="cs_sb")
        ss_sb = const.tile([128, L], BF, name="ss_sb", tag="ss_sb")
        wv_sb = const.tile([128, NE, HPG * 128], BF, name="wv_sb", tag="wv_sb")
        wv_r = d["wv"].rearrange("(ec p) f -> p ec f", p=128)
        masks_sb = const.tile([128, 4, 512], BF, name="masks_sb", tag="masks_sb")
        ones_sb = const.tile([128, 128], BF, name="ones_sb", tag="ones_sb")
        wout_sb = const.tile([128, HPG, E], BF, name="wout_sb", tag="wout_sb")

        # ---- residents ----
        q_sb = [qkres.tile([128, L], BF, name=f"q_sb{h}", tag=f"q_sb{h}")
                for h in range(HPG)]
        k_sb = [qkres.tile([128, L], BF, name=f"k_sb{h}", tag=f"k_sb{h}")
                for h in range(HPG)]
        v_sb = vres.tile([128, NLT, HPG * 128], BF, name="v_sb", tag="v_sb")
        y_sb = [yres.tile([128, L], BF, name=f"y_sb{h}", tag=f"y_sb{h}")
                for h in range(HPG)]

        # ================= phase 1: QKV projection + fused rope ============
        # v projections for lc2/lc3 are NOT done here: they are deferred into
        # phase 2 as PE filler work (attention-only stretches there are paced
        # by the scalar-engine exp, leaving the PE underfed).  Their x tiles
        # stay resident (xs bufs=32 covers lc2+lc3).
        xt_keep = {}
        with tc.tile_pool(name="psum1", bufs=1, space="PSUM") as ps1:

            def acc_tile(nm):
                return ps1.tile([128, 512], FP32, name=nm, tag="pacc", bufs=8)

            for lc in range(NLC):
                ls_lo = lc * 512
                cs_lc = cs_sb[:, ls_lo:ls_lo + 512]
                ss_lc = ss_sb[:, ls_lo:ls_lo + 512]

                xt_t = []
                for e in range(NE):
                    t = xs.tile([128, 512], BF, name=f"xt_{lc}_{e}", tag="xt")
                    eng = nc.sync if e % 2 == 0 else nc.scalar
                    if lc == 0:
                        # startup-critical: (wqk chunk e, x tile e) pairs in
                        # exact consumption order, alternating between the
                        # two hardware DMA queues
                        eng.dma_start(out=wqk_sb[:, e, :], in_=wqk_r[:, e, :])
                    eng.dma_start(
                        out=t,
                        in_=d["xt"][e * 128:(e + 1) * 128, ls_lo:ls_lo + 512])
                    xt_t.append(t)
                if lc == 0:
                    # bulk loads, ordered by first use: rope tables
                    # (~t+35us), wv (~t+40us), masks/ones (phase-2 start),
                    # wout (first projection).  wv is chunked so the e-outer
                    # v_pass can start on chunk 0 while the rest stream in.
                    nc.scalar.dma_start(out=cs_sb, in_=d["cs"])
                    nc.sync.dma_start(out=ss_sb, in_=d["ss"])
                    for e in range(NE):
                        nc.scalar.dma_start(out=wv_sb[:, e, :],
                                            in_=wv_r[:, e, :])
                    nc.sync.dma_start(
                        out=masks_sb,
                        in_=d["masks"].rearrange("p (r f) -> p r f", r=4))
                    nc.sync.dma_start(out=ones_sb, in_=d["ones"])
                    nc.sync.dma_start(
                        out=wout_sb,
                        in_=d["wout"].rearrange("(h p) f -> p h f", p=128))

                def qk_pass(halves):
                    # projection matmuls for the given f-block halves
                    # (0 = q heads, 1 = k heads); passing both interleaves
                    # them per e-chunk, which halves the weight-chunk arrival
                    # rate the PE needs (used for the DMA-bound first l-chunk)
                    acc = [acc_tile(f"p{half}_{lc}_{h}")
                           for half in halves for h in range(HPG)]
                    for e in range(NE):
                        for i, half in enumerate(halves):
                            for h in range(HPG):
                                fb = half * HPG + h
                                nc.tensor.matmul(
                                    acc[i * HPG + h],
                                    lhsT=wqk_sb[:, e, fb * 128:(fb + 1) * 128],
                                    rhs=xt_t[e],
                                    start=(e == 0), stop=(e == NE - 1))
                    return acc

                def rope_a(acc, which):
                    # a = q*ss (bf16), dst-slice = q*cs ; releases acc
                    a_t = []
                    for h in range(HPG):
                        a = atile.tile([128, 512], BF,
                                       name=f"a_{which}{h}_{lc}", tag="a")
                        nc.vector.tensor_mul(out=a, in0=acc[h], in1=ss_lc)
                        dst = (q_sb if which == "q" else k_sb)[h]
                        nc.vector.tensor_mul(
                            out=dst[:, ls_lo:ls_lo + 512], in0=acc[h], in1=cs_lc)
                        a_t.append(a)
                    return a_t

                def rope_b(a_t, which):
                    # dst -= shuf16(a)   (rope partner lives at p XOR 16)
                    for h in range(HPG):
                        b = atile.tile([128, 512], BF,
                                       name=f"b_{which}{h}_{lc}", tag="a")
                        nc.vector.stream_shuffle(out=b, in_=a_t[h], mask=SWAP16)
                        dst = (q_sb if which == "q" else k_sb)[h]
                        sl = dst[:, ls_lo:ls_lo + 512]
                        nc.vector.tensor_sub(out=sl, in0=sl, in1=b)

                def v_pass():
                    # v pass (x tiles stationary -> natural [l, d] layout);
                    # e-outer with 4 live accumulators so the first chunk's
                    # arrival unblocks the whole pass (wv streams chunk-wise)
                    accv = [acc_tile(f"pv_{lc * 4 + ls}") for ls in range(4)]
                    for e in range(NE):
                        for ls in range(4):
                            nc.tensor.matmul(
                                accv[ls],
                                lhsT=xt_t[e][:, ls * 128:(ls + 1) * 128],
                                rhs=wv_sb[:, e, :],
                                start=(e == 0), stop=(e == NE - 1))
                    for ls in range(4):
                        nc.scalar.copy(out=v_sb[:, lc * 4 + ls, :],
                                       in_=accv[ls])

                if lc == 0:
                    acc8 = qk_pass((0, 1))
                    accq, acck = acc8[:HPG], acc8[HPG:]
                    aq = rope_a(accq, "q")
                    ak = rope_a(acck, "k")
                    rope_b(aq, "q")
                    rope_b(ak, "k")
                    v_pass()
                elif lc == 1:
                    accq = qk_pass((0,))
                    aq = rope_a(accq, "q")
                    acck = qk_pass((1,))
                    rope_b(aq, "q")
                    ak = rope_a(acck, "k")
                    v_pass()
                    rope_b(ak, "k")
                else:
                    # lc2/lc3: no v pass (deferred into phase 2)
                    accq = qk_pass((0,))
                    aq = rope_a(accq, "q")
                    acck = qk_pass((1,))
                    rope_b(aq, "q")
                    ak = rope_a(acck, "k")
                    rope_b(ak, "k")
                    xt_keep[lc] = xt_t

        # ======== phase 2+3: causal attention with interleaved projection ==
        # jobs are ic-major: once all 4 heads finished l-chunk ic, that
        # chunk's output projection is emitted immediately — it fills
        # attention pipeline bubbles and spreads the output DMA.
        with tc.tile_pool(name="psum2", bufs=1, space="PSUM") as ps2:
            jobs = [(h, ic) for ic in range(NLC) for h in range(HPG)]
            steps = [(ji, jb)
                     for ji, (_h, ic) in enumerate(jobs)
                     for jb in range(4 * ic + 4)]
            LA = 3
            pss_map = {}
            psy_map = {}
            s_map = {}

            def emit_s(ji, jb):
                h, ic = jobs[ji]
                # diagonal blocks (r >= 1) have no valid columns below
                # f = 128*r: compute only the valid column range
                r = jb - 4 * ic
                lo = r * 128 if r > 0 else 0
                t = ps2.tile([128, 512], FP32, name=f"pss_{ji}_{jb}",
                             tag="pss", bufs=4)
                nc.tensor.matmul(
                    t[:, lo:],
                    lhsT=k_sb[h][:, jb * 128:(jb + 1) * 128],
                    rhs=q_sb[h][:, ic * 512 + lo:(ic + 1) * 512],
                    start=True, stop=True)
                pss_map[(ji, jb)] = t

            def emit_proj(ft, lc):
                # one [128f, 512l] block of the output projection for chunk
                # lc (all 4 heads accumulated) + its store/DMA
                po = ps2.tile([128, 512], FP32, name=f"po_{ft}_{lc}",
                              tag="po", bufs=2)
                for hh in range(HPG):
                    nc.tensor.matmul(
                        po,
                        lhsT=wout_sb[:, hh, ft * 128:(ft + 1) * 128],
                        rhs=y_sb[hh][:, lc * 512:(lc + 1) * 512],
                        start=(hh == 0), stop=(hh == HPG - 1))
                ot = outst.tile([128, 512], mybir.dt.float16,
                                name=f"ot_{ft}_{lc}", tag="ot", bufs=6)
                if ft % 2 == 0:
                    nc.vector.tensor_copy(out=ot, in_=po)
                else:
                    nc.scalar.copy(out=ot, in_=po)
                if lc == NLC - 1:
                    # tail: drain the final chunk across all three queues
                    eng = (nc.sync, nc.scalar, nc.gpsimd)[ft % 3]
                else:
                    # keep dribbled DMAs off the scalar queue: a DMA issue
                    # (~670ns) would stall the latency-critical exp stream
                    eng = nc.sync
                eng.dma_start(
                    out=d["out"][ft * 128:(ft + 1) * 128,
                                 lc * 512:(lc + 1) * 512],
                    in_=ot)

            # ---- deferred v projections for lc2/lc3, one lt-pair at a time
            # (the pair's two accumulators use the 'po' PSUM rotation)
            vacc = {}

            def v_fill(lcf, pair, e):
                if e == 0:
                    vacc[(lcf, pair)] = [
                        ps2.tile([128, 512], FP32,
                                 name=f"pvf_{lcf}_{pair}_{j}", tag="po",
                                 bufs=2)
                        for j in range(2)]
                acc = vacc[(lcf, pair)]
                for j in range(2):
                    ls = 2 * pair + j
                    nc.tensor.matmul(
                        acc[j],
                        lhsT=xt_keep[lcf][e][:, ls * 128:(ls + 1) * 128],
                        rhs=wv_sb[:, e, :],
                        start=(e == 0), stop=(e == NE - 1))
                if e == NE - 1:
                    for j in range(2):
                        ls = 2 * pair + j
                        nc.scalar.copy(out=v_sb[:, lcf * 4 + ls, :],
                                       in_=vacc[(lcf, pair)][j])
                    del vacc[(lcf, pair)]

            # FIFO PE-filler queue: (ready_idx, thunk).  v fills come first
            # (lc2 needed from step ~56, lc3 from ~108), proj blocks are
            # appended as chunks complete.  Pacing: v fills at 1/step, then
            # proj at ~0.5/step, so filler lasts through the exp-paced
            # attention stretches to the very end.
            fill_q = []
            for lcf in (2, 3):
                for pair in (0, 1):
                    for e in range(NE):
                        fill_q.append(
                            (0, lambda lcf=lcf, pair=pair, e=e:
                             v_fill(lcf, pair, e)))
            n_v = len(fill_q)
            popped = 0
            ptr = 0
            for idx, (ji, jb) in enumerate(steps):
                while ptr < len(steps) and ptr <= idx + LA:
                    emit_s(*steps[ptr])
                    ptr += 1
                h, ic = jobs[ji]
                njb = 4 * ic + 4
                if jb == 0:
                    psy_map[ji] = ps2.tile([128, 512], FP32, name=f"psy_{ji}",
                                           tag="psy", bufs=2)
                    s_map[ji] = spool.tile([128, 512], BF, name=f"s_{ji}",
                                           tag="s")
                psy = psy_map[ji]
                s_t = s_map[ji]
                pss = pss_map.pop((ji, jb))
                r = jb - 4 * ic
                lo = r * 128 if r > 0 else 0
                # block 0's exp goes straight into the S accumulator (it is
                # always full-width); later blocks exp into a scratch tile
                # that is then added to S
                if jb == 0:
                    pt = s_t
                else:
                    pt = pexp.tile([128, 512], BF, name=f"pt_{ji}_{jb}",
                                   tag="pexp")
                nc.scalar.activation(out=pt[:, lo:], in_=pss[:, lo:], func=EXP)
                if r >= 0:
                    # diagonal block: only the first 128 columns of the valid
                    # range hold the per-element triangle; the rest are all-1
                    nc.vector.tensor_mul(
                        out=pt[:, lo:lo + 128], in0=pt[:, lo:lo + 128],
                        in1=masks_sb[:, r, lo:lo + 128])
                if jb > 0:
                    nc.vector.tensor_add(out=s_t[:, lo:], in0=s_t[:, lo:],
                                         in1=pt[:, lo:])
                nc.tensor.matmul(psy[:, lo:],
                                 lhsT=v_sb[:, jb, h * 128:(h + 1) * 128],
                                 rhs=pt[:, lo:],
                                 start=(jb == 0), stop=(jb == njb - 1))
                if jb == njb - 1:
                    # one all-ones matmul: Z broadcast across partitions
                    # (allocated from the pss rotation: it frees a PSUM bank
                    # for the projection double-buffer)
                    psz = ps2.tile([128, 512], FP32, name=f"psz_{ji}",
                                   tag="pss", bufs=4)
                    nc.tensor.matmul(psz, lhsT=ones_sb, rhs=s_map.pop(ji),
                                     start=True, stop=True)
                    zv = zpool.tile([128, 512], FP32, name=f"zinv_{ji}",
                                    tag="zinv")
                    nc.vector.reciprocal_approx_fast(out=zv, in_=psz)
                    nc.vector.tensor_mul(
                        out=y_sb[h][:, ic * 512:(ic + 1) * 512],
                        in0=psy_map.pop(ji), in1=zv)
                    if h == HPG - 1:
                        # all heads done for this l-chunk: queue its
                        # projection blocks as PE filler.  5 steps of delay
                        # so the PE doesn't stall in-order behind the y
                        # finalize chain (psz -> reciprocal -> y mul on DVE).
                        fill_q.extend(
                            (idx + 5, lambda ft=ft, ic=ic: emit_proj(ft, ic))
                            for ft in range(NE))
                target = min(idx + 1, n_v) + max(
                    0, (max(0, idx + 1 - n_v) * 48) // (len(steps) - n_v))
                while (popped < target and fill_q
                       and fill_q[0][0] <= idx):
                    fill_q.pop(0)[1]()
                    popped += 1
            for _, thunk in fill_q:
                thunk()


# ------------------------------------------------------------------ host side

# row p of a head's q/k block holds original head element _PERM_IDX[p]:
# rope pair i occupies rows (q*32 + j, q*32 + 16 + j) with i = q*16 + j,
# so the rotation partner of row p is p XOR 16 (stream_shuffle-able).
_PERM_IDX = np.array(
    [2 * ((p // 32) * 16 + p % 16) + (1 if p % 32 >= 16 else 0)
     for p in range(128)])
_PAIR_IDX = (np.arange(128) // 32) * 16 + np.arange(128) % 16
_SIGN = np.where(np.arange(128) % 32 < 16, -1.0, 1.0).astype(np.float32)


def prep_in_maps(x, rope, w_attn, w_proj):
    x = np.asarray(x, np.float32)
    rope = np.asarray(rope, np.float32)
    w_attn = np.asarray(w_attn, np.float32)
    w_proj = np.asarray(w_proj, np.float32)

    sin = rope[:, :, 0]                      # [L, 64]
    cos = rope[:, :, 1]
    cs = (cos.T[_PAIR_IDX, :] * SCALE).astype(BF16)                 # [128, L]
    ss = (sin.T[_PAIR_IDX, :] * _SIGN[:, None] * SCALE).astype(BF16)

    p = np.arange(128)[:, None]
    f = np.arange(512)[None, :]
    masks = np.zeros((128, 4, 512), np.float32)
    for r in range(4):
        masks[:, r, :] = (r * 128 + p <= f).astype(np.float32)
    masks = masks.reshape(128, 4 * 512).astype(BF16)

    ones = np.ones((128, 128), np.float32).astype(BF16)

    xt_b = [np.ascontiguousarray(x[b].T).astype(BF16) for b in range(B)]

    wqk_g, wv_g, wout_g = {}, {}, {}
    for g in range(G):
        heads = [g * HPG + hl for hl in range(HPG)]
        wq = [np.ascontiguousarray(
                 w_attn[h * 128:(h + 1) * 128, :][_PERM_IDX, :].T) for h in heads]
        wk = [np.ascontiguousarray(
                 w_attn[E + h * 128:E + (h + 1) * 128, :][_PERM_IDX, :].T)
              for h in heads]
        wqk_g[g] = np.concatenate(wq + wk, axis=1).astype(BF16)        # [E, 1024]
        wv_g[g] = np.concatenate(
            [w_attn[2 * E + h * 128:2 * E + (h + 1) * 128, :].T for h in heads],
            axis=1).astype(BF16)                                        # [E, 512]
        wout_g[g] = np.ascontiguousarray(
            w_proj[:, g * 512:(g + 1) * 512].T).astype(BF16)            # [512, E]

    in_maps = []
    for c in range(NCORES):
        b, g = divmod(c, G)
        in_maps.append({
            "xt": xt_b[b],
            "wqk": wqk_g[g],
            "wv": wv_g[g],
            "wout": wout_g[g],
            "cs": cs,
            "ss": ss,
            "masks": masks,
            "ones": ones,
        })
    return in_maps


def assemble_output(results):
    out = np.zeros((B, L, E), np.float32)
    for c in range(NCORES):
        b, g = divmod(c, G)
        out[b] += results[c]["out"].T
    return out


_NC = None


def get_nc():
    global _NC
    if _NC is None:
        _NC = build_nc()
    return _NC


def run(x, rope, w_attn, w_proj, trace=False, tmpdir=None):
    nc = get_nc()
    in_maps = prep_in_maps(x, rope, w_attn, w_proj)
    kwargs = {}
    if trace:
        import sys
        import types
        from concourse import bass_utils as _bu
        try:
            from trn_agent_boot.trn_boot import _ntff_profile_via_ctypes
            hook = _ntff_profile_via_ctypes("/opt/axon/libaxon_pjrt.so")
            mod = types.ModuleType("antenv.axon_hooks")
            mod.get_axon_ntff_profile_hook = lambda: hook
            sys.modules["antenv.axon_hooks"] = mod
            _bu.upload_artifacts = lambda dd: dd
        except Exception as e:  # pragma: no cover
            print("trace hook unavailable:", e)
        kwargs = dict(trace=True, tmpdir=tmpdir)
    res = run_bass_kernel_spmd(nc, in_maps, core_ids=list(range(NCORES)), **kwargs)
    return assemble_output(res.results), res


def kernel(x, rope, w_attn, w_proj):
    out, _ = run(x, rope, w_attn, w_proj, trace=False)
    return out
